# revision 1
# baseline (speedup 1.0000x reference)
"""GPT-2 decode-step kernel for 8 Trainium2 NeuronCores (Bass/Tile).

Sharding (tensor parallel over 8 cores):
  - attention: 2 heads per core (KV cache, qkv weights split on head axis)
  - MLP: fc column-split (512 of 4096 per core), proj row-split -> partial sums
  - lm_head: vocab split (6656 padded rows of wte per core)
  - per-layer AllReduce (x2) over the [8,1024] activations; AR for embeddings;
    AR-max / AR-add for the final softmax over the full vocab.

Activations live TRANSPOSED on chip: hT[128, 64] = 8 feature-chunks x 8 tokens,
so every GEMM consumes weights in natural [in,out] layout as lhsT tiles.
"""

import sys

sys.path.insert(0, "/opt/trn_rl_repo")

import numpy as np

import concourse.bass as bass
import concourse.mybir as mybir
import concourse.tile as tile
from concourse import bacc
from concourse.bass_utils import run_bass_kernel_spmd
from concourse.masks import make_identity

F32 = mybir.dt.float32
I32 = mybir.dt.int32
AF = mybir.ActivationFunctionType
ALU = mybir.AluOpType
AX = mybir.AxisListType

# model dims
L, B, H, D, E, F, V, S = 12, 8, 16, 64, 1024, 4096, 50257, 1024
T = 1024  # 1023 cached + 1 new
NC = 8  # cores
HC = H // NC  # 2 heads per core
FC = F // NC  # 512
VS = (V + NC - 1) // NC  # 6283 vocab rows per core (last core fewer)
VPAD = 6656  # 13 * 512
NVT = VPAD // 512  # 13 n-tiles in lm head
EPS = 1e-5
NEG = -30000.0

_CACHED = {}


# ----------------------------------------------------------------------------
# device program
# ----------------------------------------------------------------------------
def _ln_transposed(nc, tc, wrk, ps_misc, ps_small, hT, w_col, b_col, ones128, eps1, out_name):
    """LayerNorm over E=1024 for hT [128, 8c x 8t] transposed layout.
    Returns xT [128, 64] normalized*w+b."""
    sq = wrk.tile([128, 64], F32, name="ln_sq", tag="ln_sq")
    nc.vector.tensor_mul(out=sq[:], in0=hT[:], in1=hT[:])
    s1 = ps_misc.tile([1, 64], F32, name="ln_s1", tag="psm")
    s2 = ps_misc.tile([1, 64], F32, name="ln_s2", tag="psm")
    nc.tensor.matmul(s1[:], lhsT=ones128[:, 0:1], rhs=hT[:], start=True, stop=True)
    nc.tensor.matmul(s2[:], lhsT=ones128[:, 0:1], rhs=sq[:], start=True, stop=True)
    # stats [1, 16]: cols 0-7 mean, 8-15 rstd
    st = wrk.tile([1, 16], F32, name="ln_st", tag="ln_st")
    # sum over chunks: view [1, 64] as (c t) -> t c, reduce inner (c, stride 8)
    nc.vector.reduce_sum(
        st[0:1, 0:8], s1[0:1, :].rearrange("p (c t) -> p t c", c=8), axis=AX.X
    )
    nc.vector.reduce_sum(
        st[0:1, 8:16], s2[0:1, :].rearrange("p (c t) -> p t c", c=8), axis=AX.X
    )
    mean = wrk.tile([1, 8], F32, name="ln_mean", tag="ln_mean")
    var = wrk.tile([1, 8], F32, name="ln_var", tag="ln_var")
    nc.vector.tensor_scalar_mul(mean[:], st[0:1, 0:8], 1.0 / E)
    nc.vector.tensor_scalar_mul(var[:], st[0:1, 8:16], 1.0 / E)  # E[x^2]
    msq = wrk.tile([1, 8], F32, name="ln_msq", tag="ln_msq")
    nc.vector.tensor_mul(out=msq[:], in0=mean[:], in1=mean[:])
    nc.vector.tensor_sub(out=var[:], in0=var[:], in1=msq[:])
    # rstd = 1/sqrt(var + eps)
    sd = wrk.tile([1, 8], F32, name="ln_sd", tag="ln_sd")
    nc.scalar.activation(sd[:], var[:], AF.Sqrt, bias=eps1[0:1, 0:1], scale=1.0)
    nc.vector.reciprocal(st[0:1, 8:16], sd[:])
    nc.vector.tensor_copy(st[0:1, 0:8], mean[:])
    # broadcast [1,16] -> [128,16] via K=1 matmul
    bc = ps_misc.tile([128, 16], F32, name="ln_bc", tag="psm")
    nc.tensor.matmul(bc[:], lhsT=ones128[0:1, 0:128], rhs=st[0:1, :], start=True, stop=True)
    bcs = wrk.tile([128, 16], F32, name="ln_bcs", tag="ln_bcs")
    nc.vector.tensor_copy(bcs[:], bc[:])
    # x = (hT - mean) * rstd ; broadcast over chunk dim with 0-stride
    xT = wrk.tile([128, 64], F32, name=out_name, tag=out_name)
    h3 = hT[:, :].rearrange("p (c t) -> p c t", c=8)
    x3 = xT[:, :].rearrange("p (c t) -> p c t", c=8)
    mb = bcs[:, None, 0:8].to_broadcast([128, 8, 8])
    rb = bcs[:, None, 8:16].to_broadcast([128, 8, 8])
    nc.vector.tensor_tensor(out=x3, in0=h3, in1=mb, op=ALU.subtract)
    nc.vector.tensor_tensor(out=x3, in0=x3, in1=rb, op=ALU.mult)
    # * w + b via replicated [128, 64] param tiles (pure DVE)
    nc.vector.tensor_mul(out=xT[:], in0=xT[:], in1=w_col)
    nc.vector.tensor_add(out=xT[:], in0=xT[:], in1=b_col)
    return xT


def build_program(variant="full"):
    nc = bacc.Bacc(None, target_bir_lowering=False, num_devices=NC)
    rg = [list(range(NC))]

    def all_reduce(op, ins, outs):
        if variant == "nocoll":
            nc.sync.dma_start(outs[0], ins[0])
        else:
            nc.gpsimd.collective_compute(
                "AllReduce", op, replica_groups=rg, ins=ins, outs=outs
            )

    # ---- dram inputs --------------------------------------------------------
    din = {}
    def inp(name, shape, dtype=F32):
        din[name] = nc.dram_tensor(name, list(shape), dtype, kind="ExternalInput")
        return din[name]

    inp("lid", (8, 1), I32)
    inp("emb_mask", (8, 1))
    inp("wpe_rep", (8, E))
    inp("wte_nat", (VPAD, E))
    inp("wte_lm", (NVT, 128, 8, 512))
    inp("vbias", (1, VPAD))
    inp("kT", (L, B, 128, T))
    inp("vP", (L, B, T, 128))
    inp("wqkv", (L, 128, 3072))
    inp("wproj", (L, 128, 1024))
    inp("wfc", (L, 128, 4096))
    inp("wmlp", (L, 128, 4096))
    inp("bqkv", (L, 128, 24))
    inp("bproj", (L, 128, 64))
    inp("bfc", (L, 128, 32))
    inp("bmlp", (L, 128, 64))
    inp("lnw1", (L, 128, 64))
    inp("lnb1", (L, 128, 64))
    inp("lnw2", (L, 128, 64))
    inp("lnb2", (L, 128, 64))
    inp("lnfw", (128, 64))
    inp("lnfb", (128, 64))
    inp("ab_rep", (128, L))
    probs_out = nc.dram_tensor("probs", [8, VPAD], F32, kind="ExternalOutput")

    with tile.TileContext(nc, num_cores=NC) as tc:
        with (
            tc.tile_pool(name="const", bufs=1) as const,
            tc.tile_pool(name="act", bufs=1) as act,
            tc.tile_pool(name="wrk", bufs=3) as wrk,
            tc.tile_pool(name="dram", bufs=4, space="DRAM") as dram,
            tc.tile_pool(name="ps_small", bufs=3, space="PSUM") as ps_small,
            tc.tile_pool(name="ps_misc", bufs=3, space="PSUM") as ps_misc,
        ):
            # ---- constants / params resident in SBUF ------------------------
            ones128 = const.tile([128, 128], F32, name="ones128")
            nc.vector.memset(ones128[:], 1.0)
            ident = const.tile([128, 128], F32, name="ident")
            make_identity(nc, ident[:])
            def load_packed(name, ccount):
                sb = const.tile([128, L * ccount], F32, name=f"{name}_sb", uniquify=False)
                nc.sync.dma_start(
                    sb[:].rearrange("p (l c) -> p l c", c=ccount),
                    din[name][:].rearrange("l p c -> p l c"),
                )
                return sb

            lnw1_sb = load_packed("lnw1", 64)
            lnb1_sb = load_packed("lnb1", 64)
            lnw2_sb = load_packed("lnw2", 64)
            lnb2_sb = load_packed("lnb2", 64)
            lnfw_sb = const.tile([128, 64], F32, name="lnfw_sb")
            nc.sync.dma_start(lnfw_sb[:], din["lnfw"][:])
            lnfb_sb = const.tile([128, 64], F32, name="lnfb_sb")
            nc.sync.dma_start(lnfb_sb[:], din["lnfb"][:])
            bqkv_sb = load_packed("bqkv", 24)
            bproj_sb = load_packed("bproj", 64)
            bfc_sb = load_packed("bfc", 32)
            bmlp_sb = load_packed("bmlp", 64)
            ab_sb = const.tile([128, L], F32, name="ab_sb")
            nc.sync.dma_start(ab_sb[:], din["ab_rep"][:])

            hT = act.tile([128, 64], F32, name="hT")
            eps1 = const.tile([1, 1], F32, name="eps1")
            nc.vector.memset(eps1[:], EPS)

            # ---- embedding --------------------------------------------------
            lid_sb = wrk.tile([8, 1], I32, name="lid_sb", bufs=1)
            nc.sync.dma_start(lid_sb[:], din["lid"][:])
            mask_sb = wrk.tile([8, 1], F32, name="mask_sb", bufs=1)
            nc.sync.dma_start(mask_sb[:], din["emb_mask"][:])
            emb = wrk.tile([8, E], F32, name="emb", bufs=1)
            nc.gpsimd.indirect_dma_start(
                out=emb[:],
                out_offset=None,
                in_=din["wte_nat"][:],
                in_offset=bass.IndirectOffsetOnAxis(ap=lid_sb[:, 0:1], axis=0),
            )
            nc.vector.tensor_scalar_mul(emb[:], emb[:], mask_sb[:, 0:1])
            wpe_sb = wrk.tile([8, E], F32, name="wpe_sb", bufs=1)
            nc.sync.dma_start(wpe_sb[:], din["wpe_rep"][:])
            nc.vector.tensor_add(out=emb[:], in0=emb[:], in1=wpe_sb[:])
            h0 = wrk.tile([128, 64], F32, name="h0", bufs=1)
            for c in range(8):
                pt = ps_misc.tile([128, 8], F32, name="emb_t", tag="psm")
                nc.tensor.transpose(pt[:], emb[0:8, 128 * c : 128 * (c + 1)], ident[0:8, 0:8])
                nc.vector.tensor_copy(h0[:, 8 * c : 8 * c + 8], pt[:])
            ar_in = dram.tile([128, 64], F32, name="ar_in_emb")
            ar_out = dram.tile([128, 64], F32, name="ar_out_emb", addr_space="Shared")
            nc.gpsimd.dma_start(ar_in[:], h0[:])
            all_reduce(ALU.add, [ar_in[:].opt()], [ar_out[:].opt()])
            nc.sync.dma_start(hT[:], ar_out[:])

            # ---- transformer layers ----------------------------------------
            with (
                tc.tile_pool(name="kpool", bufs=4) as kpool,
                tc.tile_pool(name="vpool", bufs=9) as vpool,
                tc.tile_pool(name="wq_pool", bufs=2) as wq_pool,
                tc.tile_pool(name="wp_pool", bufs=2) as wp_pool,
                tc.tile_pool(name="wf_pool", bufs=2) as wf_pool,
                tc.tile_pool(name="wm_pool", bufs=2) as wm_pool,
                tc.tile_pool(name="ps_sc", bufs=1, space="PSUM") as ps_sc_pool,
            ):
                for l in range(L):
                    with nc.named_scope(f"layer{l}"):
                        xT = _ln_transposed(
                            nc, tc, wrk, ps_misc, ps_small, hT,
                            lnw1_sb[:, 64 * l : 64 * l + 64], lnb1_sb[:, 64 * l : 64 * l + 64],
                            ones128, eps1, "x1",
                        )
                        # qkv
                        wq_sb = wq_pool.tile([128, 3072], F32, name="wq_sb")
                        nc.sync.dma_start(wq_sb[:], din["wqkv"][l])
                        ps_qkv = ps_small.tile([128, 24], F32, name="qkv_ps", tag="ps8")
                        for m in range(3):
                            for k in range(8):
                                nc.tensor.matmul(
                                    ps_qkv[:, 8 * m : 8 * m + 8],
                                    lhsT=wq_sb[:, (k * 3 + m) * 128 : (k * 3 + m + 1) * 128],
                                    rhs=xT[:, 8 * k : 8 * k + 8],
                                    start=(k == 0),
                                    stop=(k == 7),
                                    skip_group_check=True,
                                )
                        qkv_sb = wrk.tile([128, 24], F32, name="qkv_sb")
                        nc.vector.tensor_add(
                            out=qkv_sb[:], in0=ps_qkv[:], in1=bqkv_sb[:, 24 * l : 24 * l + 24]
                        )
                        qT = qkv_sb[:, 0:8]
                        kTn = qkv_sb[:, 8:16]
                        vTn = qkv_sb[:, 16:24]

                        ps_sc = ps_sc_pool.tile([16, 1024], F32, name="ps_sc")
                        vtiles = []
                        for b in range(B):
                            KT = kpool.tile([128, T], F32, name="KT")
                            nc.sync.dma_start(KT[:], din["kT"][l, b])
                            nc.vector.tensor_copy(KT[:, 0:1], kTn[:, b : b + 1])
                            VT = vpool.tile([128, 8, 128], F32, name="VT")
                            nc.sync.dma_start(
                                VT[:], din["vP"][l, b].rearrange("(c p) d -> p c d", p=128)
                            )
                            ps_vb = ps_misc.tile([1, 128], F32, name="ps_vb", tag="psm")
                            nc.tensor.transpose(
                                ps_vb[:], vTn[:, b : b + 1], ident[:, :]
                            )
                            nc.vector.tensor_copy(VT[0:1, 0, :], ps_vb[0:1, :])
                            vtiles.append(VT)
                            qzb = wrk.tile([128, 16], F32, name="qzb", tag="qzb", bufs=4)
                            nc.vector.memset(qzb[:], 0.0)
                            nc.vector.tensor_copy(
                                qzb[0:64, 2 * b : 2 * b + 1], qT[0:64, b : b + 1]
                            )
                            nc.vector.tensor_copy(
                                qzb[64:128, 2 * b + 1 : 2 * b + 2], qT[64:128, b : b + 1]
                            )
                            for n in range(2):
                                nc.tensor.matmul(
                                    ps_sc[:, 512 * n : 512 * (n + 1)],
                                    lhsT=qzb[:, :],
                                    rhs=KT[:, 512 * n : 512 * (n + 1)],
                                    start=(b == 0),
                                    stop=(b == B - 1),
                                    skip_group_check=True,
                                )
                        # softmax over free dim (1024)
                        rmax = wrk.tile([16, 1], F32, name="rmax")
                        nc.vector.reduce_max(rmax[:], ps_sc[:, :], axis=AX.X)
                        nbias = wrk.tile([16, 1], F32, name="nbias")
                        nc.vector.tensor_scalar_mul(nbias[:], rmax[:], -1.0)
                        nc.vector.tensor_add(
                            out=nbias[:], in0=nbias[:], in1=ab_sb[0:16, l : l + 1]
                        )
                        attn = wrk.tile([16, 1024], F32, name="attn", bufs=2)
                        dsum = wrk.tile([16, 1], F32, name="dsum")
                        nc.scalar.activation(
                            attn[:], ps_sc[:, :], AF.Exp, bias=nbias[:, 0:1], scale=1.0,
                            accum_out=dsum[:, 0:1],
                        )
                        rd = wrk.tile([16, 1], F32, name="rd")
                        nc.vector.reciprocal(rd[:], dsum[:])
                        nc.vector.tensor_scalar_mul(attn[:], attn[:], rd[:, 0:1])
                        # transpose attn -> aT [128, 8c, 16]
                        aT = wrk.tile([128, 8, 16], F32, name="aT")
                        pt = ps_misc.tile([128, 8, 16], F32, name="aT_ps", tag="psm")
                        for c in range(8):
                            nc.tensor.transpose(
                                pt[:, c, :], attn[:, 128 * c : 128 * (c + 1)], ident[0:16, 0:16]
                            )
                        nc.vector.tensor_copy(aT[:], pt[:])
                        # ctx
                        ctx_ps = ps_small.tile([128, 8], F32, name="ctx_ps", tag="ps8")
                        for b in range(B):
                            VT = vtiles[b]
                            for c in range(8):
                                nc.tensor.matmul(
                                    ctx_ps[0:64, b : b + 1],
                                    lhsT=VT[:, c, 0:64],
                                    rhs=aT[:, c, 2 * b : 2 * b + 1],
                                    start=(c == 0),
                                    stop=(c == 7),
                                    skip_group_check=True,
                                )
                                nc.tensor.matmul(
                                    ctx_ps[64:128, b : b + 1],
                                    lhsT=VT[:, c, 64:128],
                                    rhs=aT[:, c, 2 * b + 1 : 2 * b + 2],
                                    start=(c == 0),
                                    stop=(c == 7),
                                    skip_group_check=True,
                                )
                        ctxT = wrk.tile([128, 8], F32, name="ctxT")
                        nc.vector.tensor_copy(ctxT[:], ctx_ps[:])
                        # attn out projection (partial over this core's 128 ctx feats)
                        wp_sb = wp_pool.tile([128, 1024], F32, name="wp_sb")
                        nc.sync.dma_start(wp_sb[:], din["wproj"][l])
                        ps_pr = ps_small.tile([128, 64], F32, name="proj_ps", tag="ps8")
                        for m in range(8):
                            nc.tensor.matmul(
                                ps_pr[:, 8 * m : 8 * m + 8],
                                lhsT=wp_sb[:, 128 * m : 128 * (m + 1)], rhs=ctxT[:],
                                start=True, stop=True, skip_group_check=True,
                            )
                        apart = wrk.tile([128, 64], F32, name="apart")
                        nc.vector.tensor_add(
                            out=apart[:], in0=ps_pr[:], in1=bproj_sb[:, 64 * l : 64 * l + 64]
                        )
                        ar_in1 = dram.tile([128, 64], F32, name="ar_in1")
                        ar_out1 = dram.tile([128, 64], F32, name="ar_out1", addr_space="Shared")
                        nc.gpsimd.dma_start(ar_in1[:], apart[:])
                        all_reduce(ALU.add, [ar_in1[:].opt()], [ar_out1[:].opt()])
                        ar_sb1 = wrk.tile([128, 64], F32, name="ar_sb1")
                        nc.sync.dma_start(ar_sb1[:], ar_out1[:])
                        nc.vector.tensor_add(out=hT[:], in0=hT[:], in1=ar_sb1[:])

                        # MLP
                        x2T = _ln_transposed(
                            nc, tc, wrk, ps_misc, ps_small, hT,
                            lnw2_sb[:, 64 * l : 64 * l + 64], lnb2_sb[:, 64 * l : 64 * l + 64],
                            ones128, eps1, "x2",
                        )
                        wf_sb = wf_pool.tile([128, 4096], F32, name="wf_sb")
                        nc.sync.dma_start(wf_sb[:], din["wfc"][l])
                        ps_fc = ps_small.tile([128, 32], F32, name="fc_ps", tag="ps8")
                        for m in range(4):
                            for k in range(8):
                                nc.tensor.matmul(
                                    ps_fc[:, 8 * m : 8 * m + 8],
                                    lhsT=wf_sb[:, (k * 4 + m) * 128 : (k * 4 + m + 1) * 128],
                                    rhs=x2T[:, 8 * k : 8 * k + 8],
                                    start=(k == 0), stop=(k == 7),
                                    skip_group_check=True,
                                )
                        gpre = wrk.tile([128, 32], F32, name="gpre")
                        nc.vector.tensor_add(
                            out=gpre[:], in0=ps_fc[:], in1=bfc_sb[:, 32 * l : 32 * l + 32]
                        )
                        gT = wrk.tile([128, 32], F32, name="gT")
                        nc.scalar.activation(gT[:], gpre[:], AF.Gelu_apprx_tanh)
                        wm_sb = wm_pool.tile([128, 4096], F32, name="wm_sb")
                        nc.sync.dma_start(wm_sb[:], din["wmlp"][l])
                        ps_ml = ps_small.tile([128, 64], F32, name="mlp_ps", tag="ps8")
                        for m in range(8):
                            for k in range(4):
                                nc.tensor.matmul(
                                    ps_ml[:, 8 * m : 8 * m + 8],
                                    lhsT=wm_sb[:, (k * 8 + m) * 128 : (k * 8 + m + 1) * 128],
                                    rhs=gT[:, 8 * k : 8 * k + 8],
                                    start=(k == 0), stop=(k == 3),
                                    skip_group_check=True,
                                )
                        mpart = wrk.tile([128, 64], F32, name="mpart")
                        nc.vector.tensor_add(
                            out=mpart[:], in0=ps_ml[:], in1=bmlp_sb[:, 64 * l : 64 * l + 64]
                        )
                        ar_in2 = dram.tile([128, 64], F32, name="ar_in2")
                        ar_out2 = dram.tile([128, 64], F32, name="ar_out2", addr_space="Shared")
                        nc.gpsimd.dma_start(ar_in2[:], mpart[:])
                        all_reduce(ALU.add, [ar_in2[:].opt()], [ar_out2[:].opt()])
                        ar_sb2 = wrk.tile([128, 64], F32, name="ar_sb2")
                        nc.sync.dma_start(ar_sb2[:], ar_out2[:])
                        nc.vector.tensor_add(out=hT[:], in0=hT[:], in1=ar_sb2[:])

            # ---- final LN + lm head ----------------------------------------
            with (
                tc.tile_pool(name="lm_pool", bufs=3) as lm_pool,
                tc.tile_pool(name="lg_pool", bufs=1) as lg_pool,
                tc.tile_pool(name="ps_lm", bufs=2, space="PSUM") as ps_lm,
            ):
                xfT = _ln_transposed(
                    nc, tc, wrk, ps_misc, ps_small, hT,
                    lnfw_sb[:, 0:64], lnfb_sb[:, 0:64], ones128, eps1, "xf",
                )
                ones_row = lg_pool.tile([1, 8], F32, name="ones_row")
                nc.vector.memset(ones_row[:], 1.0)
                vbias_sb = lg_pool.tile([1, VPAD], F32, name="vbias_sb")
                nc.sync.dma_start(vbias_sb[:], din["vbias"][:])
                logits = lg_pool.tile([8, VPAD], F32, name="logits")
                for nt in range(NVT):
                    wl_sb = lm_pool.tile([128, 8, 512], F32, name="wl_sb")
                    nc.sync.dma_start(wl_sb[:], din["wte_lm"][nt])
                    ps = ps_lm.tile([8, 512], F32, name="lg_ps")
                    for k in range(8):
                        nc.tensor.matmul(
                            ps[:], lhsT=xfT[:, 8 * k : 8 * k + 8], rhs=wl_sb[:, k, :],
                            start=(k == 0), stop=False,
                        )
                    nc.tensor.matmul(
                        ps[:], lhsT=ones_row[0:1, :],
                        rhs=vbias_sb[0:1, 512 * nt : 512 * (nt + 1)],
                        start=False, stop=True,
                    )
                    nc.vector.tensor_copy(logits[:, 512 * nt : 512 * (nt + 1)], ps[:])
                lmax = wrk.tile([8, 1], F32, name="lmax")
                nc.vector.reduce_max(lmax[:], logits[:], axis=AX.X)
                mx_in = dram.tile([8, 8], F32, name="mx_in")
                mx_out = dram.tile([8, 8], F32, name="mx_out", addr_space="Shared")
                mx_sb = wrk.tile([8, 8], F32, name="mx_sb")
                nc.vector.tensor_copy(mx_sb[:], lmax[:, 0:1].to_broadcast([8, 8]))
                nc.gpsimd.dma_start(mx_in[:], mx_sb[:])
                all_reduce(ALU.max, [mx_in[:].opt()], [mx_out[:].opt()])
                gmax = wrk.tile([8, 8], F32, name="gmax")
                nc.sync.dma_start(gmax[:], mx_out[:])
                ngmax = wrk.tile([8, 1], F32, name="ngmax")
                nc.vector.tensor_scalar_mul(ngmax[:], gmax[:, 0:1], -1.0)
                esum = wrk.tile([8, 1], F32, name="esum")
                nc.scalar.activation(
                    logits[:], logits[:], AF.Exp, bias=ngmax[:, 0:1], scale=1.0,
                    accum_out=esum[:, 0:1],
                )
                sm_in = dram.tile([8, 8], F32, name="sm_in")
                sm_out = dram.tile([8, 8], F32, name="sm_out", addr_space="Shared")
                sm_sb = wrk.tile([8, 8], F32, name="sm_sb")
                nc.vector.tensor_copy(sm_sb[:], esum[:, 0:1].to_broadcast([8, 8]))
                nc.gpsimd.dma_start(sm_in[:], sm_sb[:])
                all_reduce(ALU.add, [sm_in[:].opt()], [sm_out[:].opt()])
                gsum = wrk.tile([8, 8], F32, name="gsum")
                nc.sync.dma_start(gsum[:], sm_out[:])
                rgs = wrk.tile([8, 1], F32, name="rgs")
                nc.vector.reciprocal(rgs[:], gsum[:, 0:1])
                nc.vector.tensor_scalar_mul(logits[:], logits[:], rgs[:, 0:1])
                nc.sync.dma_start(probs_out[:], logits[:])

    nc.finalize()
    return nc


# ----------------------------------------------------------------------------
# host-side packing
# ----------------------------------------------------------------------------
def _pack_inputs(inputs):
    f = lambda x: np.ascontiguousarray(np.asarray(x), dtype=np.float32)
    input_ids = np.asarray(inputs["input_ids"])
    k_cache = f(inputs["k_cache"])
    v_cache = f(inputs["v_cache"])
    wte = f(inputs["wte"])
    wpe = f(inputs["wpe"])
    c_attn_w = f(inputs["c_attn_w"])
    c_attn_b = f(inputs["c_attn_b"])
    attn_proj_w = f(inputs["attn_proj_w"])
    attn_proj_b = f(inputs["attn_proj_b"])
    fc_w = f(inputs["fc_w"])
    fc_b = f(inputs["fc_b"])
    mlp_proj_w = f(inputs["mlp_proj_w"])
    mlp_proj_b = f(inputs["mlp_proj_b"])
    ln1_w, ln1_b = f(inputs["ln1_w"]), f(inputs["ln1_b"])
    ln2_w, ln2_b = f(inputs["ln2_w"]), f(inputs["ln2_b"])
    lnf_w, lnf_b = f(inputs["lnf_w"]), f(inputs["lnf_b"])
    attn_bias = f(inputs["attn_bias"])

    ids = np.asarray(input_ids[:, -1]).astype(np.int64)  # [8]

    def rep_feat(vec):
        # vec [nchunk*128] feature-major -> [128, nchunk*8]: out[p, 8c+t] = vec[128c+p]
        nch = vec.shape[-1] // 128
        v = vec.reshape(nch, 128).T  # [128, nch]
        return np.ascontiguousarray(np.repeat(v[:, :, None], 8, axis=2).reshape(128, nch * 8))
    lnT = lambda v: np.ascontiguousarray(v.reshape(-1, 8, 128).transpose(0, 2, 1))  # [L,128,8]

    in_maps = []
    valids = []
    for c in range(NC):
        m = {}
        h0, h1 = c * HC, c * HC + HC  # head slice
        f0, f1 = c * FC, (c + 1) * FC  # mlp slice
        v0 = c * VS
        v1 = min(V, v0 + VS)
        valid = v1 - v0
        valids.append(valid)

        # embedding shard
        lid = np.clip(ids - v0, 0, VPAD - 1).astype(np.int32).reshape(8, 1)
        msk = ((ids >= v0) & (ids < v1)).astype(np.float32).reshape(8, 1)
        m["lid"], m["emb_mask"] = lid, msk
        m["wpe_rep"] = np.broadcast_to(
            wpe[S - 1] if c == 0 else np.zeros(E, np.float32), (8, E)
        ).copy()
        wslice = np.zeros((VPAD, E), np.float32)
        wslice[:valid] = wte[v0:v1]
        m["wte_nat"] = wslice
        # lm head tiles: [nt, p, k, c] with block (k, nt512) = wteT[k*128+p, nt*512+c]
        wteT = wslice.T  # [E, VPAD]
        wlm = wteT.reshape(8, 128, NVT, 512).transpose(2, 1, 0, 3)  # [nt, p, k, c]
        m["wte_lm"] = np.ascontiguousarray(wlm)
        vb = np.zeros((1, VPAD), np.float32)
        vb[0, valid:] = NEG
        m["vbias"] = vb

        # kv cache: kT [L,B,128,T] (t=T-1 zero), vP [L,B,T,128]
        kc = k_cache[:, :, h0:h1]  # [L,B,2,1023,64]
        kT = np.zeros((L, B, 128, T), np.float32)
        kT[:, :, :, 1:] = kc.transpose(0, 1, 2, 4, 3).reshape(L, B, 128, T - 1)
        m["kT"] = kT
        vc = v_cache[:, :, h0:h1]  # [L,B,2,1023,64]
        vP = np.zeros((L, B, T, 128), np.float32)
        vP[:, :, 1:] = vc.transpose(0, 1, 3, 2, 4).reshape(L, B, T - 1, 128)
        m["vP"] = vP

        # weights
        wq = np.empty((L, 128, 3072), np.float32)
        bq = np.empty((L, 128, 24), np.float32)
        for l in range(L):
            qw = c_attn_w[l][:, h0 * D : h1 * D] * 0.125  # fold 1/sqrt(D)
            kw = c_attn_w[l][:, E + h0 * D : E + h1 * D]
            vw = c_attn_w[l][:, 2 * E + h0 * D : 2 * E + h1 * D]
            Wl = np.stack([qw, kw, vw], axis=1)           # [E, 3, 128]
            # block (k, m)[p, col]  -> free (k*3+m)*128+col
            wq[l] = Wl.reshape(8, 128, 3, 128).transpose(1, 0, 2, 3).reshape(128, 3072)
            bvals = np.stack([
                c_attn_b[l][h0 * D : h1 * D] * 0.125,
                c_attn_b[l][E + h0 * D : E + h1 * D],
                c_attn_b[l][2 * E + h0 * D : 2 * E + h1 * D],
            ])  # [3, 128]
            bq[l] = np.repeat(bvals, 8, axis=0).T.reshape(128, 24, order="F")
        m["wqkv"], m["bqkv"] = np.ascontiguousarray(wq), np.ascontiguousarray(bq)

        m["wproj"] = np.ascontiguousarray(attn_proj_w[:, h0 * D : h1 * D, :])  # [L,128,1024]
        m["bproj"] = np.stack([rep_feat(attn_proj_b[l] / NC) for l in range(L)])

        wf = np.empty((L, 128, 4096), np.float32)
        for l in range(L):
            Wl = fc_w[l][:, f0:f1]  # [E, 512]
            wf[l] = (
                Wl.reshape(8, 128, 4, 128).transpose(1, 0, 2, 3).reshape(128, 4096)
            )
        m["wfc"] = np.ascontiguousarray(wf)
        m["bfc"] = np.stack([rep_feat(fc_b[l, f0:f1]) for l in range(L)])

        wm = np.empty((L, 128, 4096), np.float32)
        for l in range(L):
            Wl = mlp_proj_w[l][f0:f1, :]  # [512, 1024]
            wm[l] = (
                Wl.reshape(4, 128, 8, 128).transpose(1, 0, 2, 3).reshape(128, 4096)
            )
        m["wmlp"] = np.ascontiguousarray(wm)
        m["bmlp"] = np.stack([rep_feat(mlp_proj_b[l] / NC) for l in range(L)])

        m["lnw1"] = np.stack([rep_feat(ln1_w[l]) for l in range(L)])
        m["lnb1"] = np.stack([rep_feat(ln1_b[l]) for l in range(L)])
        m["lnw2"] = np.stack([rep_feat(ln2_w[l]) for l in range(L)])
        m["lnb2"] = np.stack([rep_feat(ln2_b[l]) for l in range(L)])
        m["lnfw"] = rep_feat(lnf_w)
        m["lnfb"] = rep_feat(lnf_b)
        m["ab_rep"] = np.ascontiguousarray(
            np.broadcast_to(attn_bias[None, :], (128, L))
        )
        in_maps.append(m)
    return in_maps, valids


def kernel(**inputs) -> np.ndarray:
    if "nc" not in _CACHED:
        _CACHED["nc"] = build_program()
    nc = _CACHED["nc"]
    in_maps, valids = _pack_inputs(inputs)
    import os
    trace = os.environ.get("BASS_TRACE", "0") == "1"
    res = run_bass_kernel_spmd(nc, in_maps, core_ids=list(range(NC)), trace=trace)
    if res.exec_time_ns is not None:
        print(f"HW exec time: {res.exec_time_ns} ns")
        if res.instructions_and_trace:
            print(f"trace: {res.instructions_and_trace[1]}")
    _CACHED["last_res"] = res
    parts = [res.results[c]["probs"][:, : valids[c]] for c in range(NC)]
    return np.ascontiguousarray(np.concatenate(parts, axis=1), dtype=np.float32)

